# revision 27
# baseline (speedup 1.0000x reference)
"""Channel-attention module kernel for 8 Trainium2 NeuronCores.

reference semantics (B=2, C=128, N=D*H*W=147456):
    q = x.reshape(B, C, N)
    energy = q @ q^T                  # [B, C, C]
    attn = softmax(rowmax(energy) - energy, axis=-1)
          = softmax(-energy, axis=-1)             (rowmax shift is a no-op)
    out = attn @ q
    return x + gamma * out

Sharding: sequence-parallel over N. Core r owns columns
[r*N/8, (r+1)*N/8) of q for both batches. Each core computes a partial
energy (contraction over its local n), a per-batch AllReduce sums the
tiny [C, C] energy across the 8 cores, each core then computes the
softmax redundantly and applies the attention to its local columns.

Pipelining: energy(b0) -> AR(b0) overlaps energy(b1); AR(b1) overlaps
phase2(b0).

Precision split:
  - energy contraction: true fp32 (softmax argmin gaps as small as 0.03
    on these inputs; one argmin flip alone is ~5% global rel err).
  - phase 2 (attn apply): bf16. The residual is folded into the
    attention matrix (attn_s = gamma/Z * P + I; P's diagonal is exactly
    0 because the energy diagonal ~ +N dominates), so phase 2 is
    out = attn_s @ q with q rounded to bf16 — error is linear, ~0.4%,
    far inside the 2e-2 gate. This makes phase-2 matmuls 4x faster than
    fp32 and lets the fp32 x chunks be freed after phase 1: x lives in
    a small fp32 ring; a resident bf16 copy (cast on the idle ScalarE
    during phase 1) feeds phase 2.
"""

import sys

sys.path.insert(0, "/opt/trn_rl_repo")

import numpy as np

B, C = 2, 128
D, H, W = 16, 96, 96
N = D * H * W  # 147456
NCORES = 8
NLOC = N // NCORES  # 18432
CHUNK = 2048
NCHUNK = NLOC // CHUNK  # 9
OTILE = 512
PIPE = 3  # transposes emitted ahead of their matmul (keeps PE fed)

_compiled = {}


def _log(msg):
    import time as _t
    print(f"[kernel {_t.strftime('%H:%M:%S')}] {msg}", flush=True)


def _build():
    import concourse.bacc as bacc
    import concourse.tile as tile
    import concourse.mybir as mybir

    _log("build start")

    f32 = mybir.dt.float32
    f16 = mybir.dt.float16
    bf16 = mybir.dt.bfloat16
    nc = bacc.Bacc("TRN2", target_bir_lowering=False, debug=False,
                   num_devices=NCORES)

    x_d = nc.dram_tensor("x", [B, C, NLOC], f32, kind="ExternalInput").ap()
    g_d = nc.dram_tensor("gamma_col", [C, 1], f32, kind="ExternalInput").ap()
    id_d = nc.dram_tensor("ident", [C, C], f32, kind="ExternalInput").ap()
    o_d = nc.dram_tensor("out", [B, C, NLOC], f16, kind="ExternalOutput").ap()

    with tile.TileContext(nc) as tc:
        with (
            tc.tile_pool(name="xring", bufs=8) as xp,
            tc.tile_pool(name="xb16", bufs=B * NCHUNK) as xbp,
            tc.tile_pool(name="qt", bufs=6) as qtp,
            tc.tile_pool(name="tps", bufs=3, space="PSUM") as tps,
            tc.tile_pool(name="eps", bufs=2, space="PSUM") as eps,
            tc.tile_pool(name="ops", bufs=3, space="PSUM") as ops,
            tc.tile_pool(name="misc", bufs=1) as mp,
            tc.tile_pool(name="ost", bufs=3) as ostp,
            tc.tile_pool(name="dram", bufs=1, space="DRAM") as dramp,
        ):
            ident = mp.tile([C, C], f32, name="ident_sb")
            nc.sync.dma_start(ident[:], id_d[:])
            gcol = mp.tile([C, 1], f32, name="gcol")
            nc.sync.dma_start(gcol[:], g_d[:])

            # Warm-up collective: the FIRST collective on this runtime pays
            # a ~45us ncfw cold-start (hw-measured); later ones hit the
            # ~10us floor. Fire a tiny dummy AllReduce immediately so the
            # real per-batch AllReduces run warm.
            w_in = dramp.tile([C, 1], f32, name="w_in")
            w_out = dramp.tile([C, 1], f32, name="w_out", addr_space="Shared")
            nc.gpsimd.dma_start(w_in[:], gcol[:])
            nc.gpsimd.collective_compute(
                "AllReduce", mybir.AluOpType.add,
                replica_groups=[list(range(NCORES))],
                ins=[w_in.opt()], outs=[w_out.opt()],
            )

            xb16 = [[xbp.tile([C, CHUNK], bf16, name=f"xb_{b}_{k}", tag="xb")
                     for k in range(NCHUNK)] for b in range(B)]

            # ---- phase 1 + per-batch AllReduce ----
            ntile_c = CHUNK // C  # 16 n-tiles of 128 per chunk
            ntile = NCHUNK * ntile_c  # 144 per batch
            E_sb = []
            for b in range(B):
                e_ps = eps.tile([C, C], f32, name=f"e_ps{b}", tag="e")
                pend = []
                mm = 0

                def flush(e_ps=e_ps):
                    nonlocal mm
                    qt = pend.pop(0)
                    nc.tensor.matmul(e_ps[:], qt[:], qt[:],
                                     start=(mm == 0), stop=(mm == ntile - 1))
                    mm += 1

                for k in range(NCHUNK):
                    xt = xp.tile([C, CHUNK], f32, name=f"x_{b}_{k}", tag="x")
                    src = x_d[b, :, k * CHUNK:(k + 1) * CHUNK]
                    if b == 0 and k == 0:
                        # split the very first load so PE can start early
                        for s in range(2):
                            nc.sync.dma_start(
                                xt[:, s * 1024:(s + 1) * 1024],
                                x_d[0, :, s * 1024:(s + 1) * 1024])
                    else:
                        nc.sync.dma_start(xt[:], src)
                    for j in range(ntile_c):
                        t = k * ntile_c + j
                        tp = tps.tile([C, C], f32, name=f"tp_{b}_{t}",
                                      tag="tp")
                        nc.tensor.transpose(
                            tp[:], xt[:, j * C:(j + 1) * C], ident[:])
                        qt = qtp.tile([C, C], f32, name=f"qt_{b}_{t}",
                                      tag="qt")
                        nc.vector.tensor_copy(qt[:], tp[:])
                        pend.append(qt)
                        if len(pend) > PIPE:
                            flush()
                    # bf16 copy for phase 2 (ScalarE is idle in phase 1);
                    # after this the fp32 ring slot can be reused.
                    nc.scalar.copy(xb16[b][k][:], xt[:])
                while pend:
                    flush()
                e_cat = mp.tile([C, C], f32, name=f"e_cat{b}")
                nc.vector.tensor_copy(e_cat[:], e_ps[:])

                ar_in = dramp.tile([C, C], f32, name=f"ar_in{b}")
                ar_out = dramp.tile([C, C], f32, name=f"ar_out{b}",
                                    addr_space="Shared")
                # bounce DMAs on GPSIMD/SWDGE: the HWDGE (sync) ring is
                # strictly FIFO, so a collective-gated load there would
                # block all later chunk loads / output stores.
                nc.gpsimd.dma_start(ar_in[:], e_cat[:])
                nc.gpsimd.collective_compute(
                    "AllReduce", mybir.AluOpType.add,
                    replica_groups=[list(range(NCORES))],
                    ins=[ar_in.opt()], outs=[ar_out.opt()],
                )
                e_red = mp.tile([C, C], f32, name=f"e_red{b}")
                nc.gpsimd.dma_start(e_red[:], ar_out[:])
                E_sb.append(e_red)

            # ---- phase 2: softmax + apply, per batch ----
            def emit_softmax(b):
                E_b = E_sb[b][:]
                mcol = mp.tile([C, 1], f32, name=f"mcol{b}")
                nc.vector.tensor_reduce(mcol[:], E_b, axis=mybir.AxisListType.X,
                                        op=mybir.AluOpType.min)
                P_b = mp.tile([C, C], f32, name=f"P{b}")
                zcol = mp.tile([C, 1], f32, name=f"zcol{b}")
                # P = exp(min_row - E), zcol = rowsum(P); exponents <= 0.
                # P's diagonal is exp(min - ~+147000) == 0 exactly.
                nc.scalar.activation(P_b[:], E_b,
                                     mybir.ActivationFunctionType.Exp,
                                     bias=mcol[:], scale=-1.0,
                                     accum_out=zcol[:])
                rz = mp.tile([C, 1], f32, name=f"rz{b}")
                nc.vector.reciprocal(rz[:], zcol[:])
                scol = mp.tile([C, 1], f32, name=f"scol{b}")
                nc.vector.tensor_tensor(scol[:], rz[:], gcol[:],
                                        op=mybir.AluOpType.mult)
                # attn_s = (gamma/Z) * P + I  -> matmul computes x + gamma*attn@q
                nc.vector.tensor_scalar_mul(P_b[:], P_b[:], scol[:])
                nc.vector.tensor_add(P_b[:], P_b[:], ident[:])
                tp2 = tps.tile([C, C], f32, name=f"tpP{b}", tag="tp")
                nc.tensor.transpose(tp2[:], P_b[:], ident[:])
                attnT = mp.tile([C, C], bf16, name=f"attnT{b}")
                nc.vector.tensor_copy(attnT[:], tp2[:])  # fp32 psum -> bf16
                return attnT

            def emit_apply_chunk(b, attnT, k):
                ost = ostp.tile([C, CHUNK], f16, name=f"ost_{b}_{k}",
                                tag="ost")
                for j in range(CHUNK // OTILE):
                    op = ops.tile([C, OTILE], f32, name=f"op_{b}_{k}_{j}",
                                  tag="op")
                    nc.tensor.matmul(
                        op[:], attnT[:],
                        xb16[b][k][:, j * OTILE:(j + 1) * OTILE],
                        start=True, stop=True)
                    dst = ost[:, j * OTILE:(j + 1) * OTILE]
                    if j % 2 == 0:
                        nc.vector.tensor_copy(dst, op[:])
                    else:
                        nc.scalar.copy(dst, op[:])
                nc.sync.dma_start(o_d[b, :, k * CHUNK:(k + 1) * CHUNK],
                                  ost[:])

            for b in range(B):
                attnT = emit_softmax(b)
                for k in range(NCHUNK):
                    emit_apply_chunk(b, attnT, k)

    _log("tile context done; bacc compile start")
    nc.compile()
    _log("bacc compile done")
    return nc


def _get_nc():
    if "nc" not in _compiled:
        _compiled["nc"] = _build()
    return _compiled["nc"]


def kernel(x, gamma, _trace=False, _tmpdir=None):
    from concourse import bass_utils

    x = np.ascontiguousarray(np.asarray(x), dtype=np.float32)
    gamma = np.asarray(gamma, dtype=np.float32)
    q = x.reshape(B, C, N)
    gcol = np.full((C, 1), gamma[0], dtype=np.float32)
    ident = np.eye(C, dtype=np.float32)

    in_maps = []
    for r in range(NCORES):
        in_maps.append({
            "x": np.ascontiguousarray(q[:, :, r * NLOC:(r + 1) * NLOC]),
            "gamma_col": gcol,
            "ident": ident,
        })

    nc = _get_nc()
    _log("launching run_bass_kernel_spmd")
    res = bass_utils.run_bass_kernel_spmd(
        nc, in_maps, core_ids=list(range(NCORES)), trace=_trace,
        tmpdir=_tmpdir)
    outs = [res.results[r]["out"] for r in range(NCORES)]
    full = np.concatenate(outs, axis=2).astype(np.float32)
    full = full.reshape(B, C, D, H, W)
    if _trace:
        return full.astype(np.float32, copy=False), res
    return full.astype(np.float32, copy=False)


# revision 28
# speedup vs baseline: 1.0161x; 1.0161x over previous
"""Channel-attention module kernel for 8 Trainium2 NeuronCores.

reference semantics (B=2, C=128, N=D*H*W=147456):
    q = x.reshape(B, C, N)
    energy = q @ q^T                  # [B, C, C]
    attn = softmax(rowmax(energy) - energy, axis=-1)
          = softmax(-energy, axis=-1)             (rowmax shift is a no-op)
    out = attn @ q
    return x + gamma * out

Sharding: sequence-parallel over N. Core r owns columns
[r*N/8, (r+1)*N/8) of q for both batches. Each core computes a partial
energy (contraction over its local n), a per-batch AllReduce sums the
tiny [C, C] energy across the 8 cores, each core then computes the
softmax redundantly and applies the attention to its local columns.

Pipelining: energy(b0) -> AR(b0) overlaps energy(b1); AR(b1) overlaps
phase2(b0).

Precision split:
  - energy contraction: true fp32 (softmax argmin gaps as small as 0.03
    on these inputs; one argmin flip alone is ~5% global rel err).
  - phase 2 (attn apply): bf16. The residual is folded into the
    attention matrix (attn_s = gamma/Z * P + I; P's diagonal is exactly
    0 because the energy diagonal ~ +N dominates), so phase 2 is
    out = attn_s @ q with q rounded to bf16 — error is linear, ~0.4%,
    far inside the 2e-2 gate. This makes phase-2 matmuls 4x faster than
    fp32 and lets the fp32 x chunks be freed after phase 1: x lives in
    a small fp32 ring; a resident bf16 copy (cast on the idle ScalarE
    during phase 1) feeds phase 2.
"""

import sys

sys.path.insert(0, "/opt/trn_rl_repo")

import numpy as np

B, C = 2, 128
D, H, W = 16, 96, 96
N = D * H * W  # 147456
NCORES = 8
NLOC = N // NCORES  # 18432
CHUNK = 2048
NCHUNK = NLOC // CHUNK  # 9
OTILE = 512
PIPE = 3  # transposes emitted ahead of their matmul (keeps PE fed)

_compiled = {}


def _log(msg):
    import time as _t
    print(f"[kernel {_t.strftime('%H:%M:%S')}] {msg}", flush=True)


def _build():
    import concourse.bacc as bacc
    import concourse.tile as tile
    import concourse.mybir as mybir

    _log("build start")

    f32 = mybir.dt.float32
    f16 = mybir.dt.float16
    bf16 = mybir.dt.bfloat16
    nc = bacc.Bacc("TRN2", target_bir_lowering=False, debug=False,
                   num_devices=NCORES)

    x_d = nc.dram_tensor("x", [B, C, NLOC], f32, kind="ExternalInput").ap()
    g_d = nc.dram_tensor("gamma_col", [C, 1], f32, kind="ExternalInput").ap()
    id_d = nc.dram_tensor("ident", [C, C], f32, kind="ExternalInput").ap()
    o_d = nc.dram_tensor("out", [B, C, NLOC], f16, kind="ExternalOutput").ap()

    with tile.TileContext(nc) as tc:
        with (
            tc.tile_pool(name="xring", bufs=8) as xp,
            tc.tile_pool(name="xb16", bufs=B * NCHUNK) as xbp,
            tc.tile_pool(name="qt", bufs=6) as qtp,
            tc.tile_pool(name="tps", bufs=3, space="PSUM") as tps,
            tc.tile_pool(name="eps", bufs=2, space="PSUM") as eps,
            tc.tile_pool(name="ops", bufs=3, space="PSUM") as ops,
            tc.tile_pool(name="misc", bufs=1) as mp,
            tc.tile_pool(name="ost", bufs=3) as ostp,
            tc.tile_pool(name="dram", bufs=1, space="DRAM") as dramp,
        ):
            ident = mp.tile([C, C], f32, name="ident_sb")
            nc.sync.dma_start(ident[:], id_d[:])
            gcol = mp.tile([C, 1], f32, name="gcol")
            nc.sync.dma_start(gcol[:], g_d[:])

            # Warm-up collective: the FIRST collective on this runtime pays
            # a ~45us ncfw cold-start (hw-measured); later ones hit the
            # ~10us floor. Fire a tiny dummy AllReduce immediately so the
            # real per-batch AllReduces run warm.
            w_in = dramp.tile([C, 1], f32, name="w_in")
            w_out = dramp.tile([C, 1], f32, name="w_out", addr_space="Shared")
            nc.gpsimd.dma_start(w_in[:], gcol[:])
            nc.gpsimd.collective_compute(
                "AllReduce", mybir.AluOpType.add,
                replica_groups=[list(range(NCORES))],
                ins=[w_in.opt()], outs=[w_out.opt()],
            )

            xb16 = [[xbp.tile([C, CHUNK], bf16, name=f"xb_{b}_{k}", tag="xb")
                     for k in range(NCHUNK)] for b in range(B)]

            # ---- phase 1 + per-batch AllReduce ----
            ntile_c = CHUNK // C  # 16 n-tiles of 128 per chunk
            ntile = NCHUNK * ntile_c  # 144 per batch
            E_sb = []
            for b in range(B):
                e_ps = eps.tile([C, C], f32, name=f"e_ps{b}", tag="e")
                pend = []
                mm = 0

                def flush(e_ps=e_ps):
                    nonlocal mm
                    qt = pend.pop(0)
                    nc.tensor.matmul(e_ps[:], qt[:], qt[:],
                                     start=(mm == 0), stop=(mm == ntile - 1))
                    mm += 1

                for k in range(NCHUNK):
                    xt = xp.tile([C, CHUNK], f32, name=f"x_{b}_{k}", tag="x")
                    src = x_d[b, :, k * CHUNK:(k + 1) * CHUNK]
                    if b == 0 and k == 0:
                        # split the very first load so PE can start early
                        for s in range(2):
                            nc.sync.dma_start(
                                xt[:, s * 1024:(s + 1) * 1024],
                                x_d[0, :, s * 1024:(s + 1) * 1024])
                    else:
                        nc.sync.dma_start(xt[:], src)
                    for j in range(ntile_c):
                        t = k * ntile_c + j
                        tp = tps.tile([C, C], f32, name=f"tp_{b}_{t}",
                                      tag="tp")
                        nc.tensor.transpose(
                            tp[:], xt[:, j * C:(j + 1) * C], ident[:])
                        qt = qtp.tile([C, C], f32, name=f"qt_{b}_{t}",
                                      tag="qt")
                        nc.vector.tensor_copy(qt[:], tp[:])
                        pend.append(qt)
                        if len(pend) > PIPE:
                            flush()
                    # bf16 copy for phase 2 (ScalarE is idle in phase 1);
                    # after this the fp32 ring slot can be reused.
                    nc.scalar.copy(xb16[b][k][:], xt[:])
                while pend:
                    flush()
                e_cat = mp.tile([C, C], f32, name=f"e_cat{b}")
                nc.vector.tensor_copy(e_cat[:], e_ps[:])

                ar_in = dramp.tile([C, C], f32, name=f"ar_in{b}")
                ar_out = dramp.tile([C, C], f32, name=f"ar_out{b}",
                                    addr_space="Shared")
                # bounce DMAs on GPSIMD/SWDGE: the HWDGE (sync) ring is
                # strictly FIFO, so a collective-gated load there would
                # block all later chunk loads / output stores.
                nc.gpsimd.dma_start(ar_in[:], e_cat[:])
                nc.gpsimd.collective_compute(
                    "AllReduce", mybir.AluOpType.add,
                    replica_groups=[list(range(NCORES))],
                    ins=[ar_in.opt()], outs=[ar_out.opt()],
                )
                e_red = mp.tile([C, C], f32, name=f"e_red{b}")
                nc.gpsimd.dma_start(e_red[:], ar_out[:])
                E_sb.append(e_red)

            # ---- phase 2: softmax + apply, per batch ----
            def emit_softmax(b):
                E_b = E_sb[b][:]
                mcol = mp.tile([C, 1], f32, name=f"mcol{b}")
                nc.vector.tensor_reduce(mcol[:], E_b, axis=mybir.AxisListType.X,
                                        op=mybir.AluOpType.min)
                P_b = mp.tile([C, C], f32, name=f"P{b}")
                zcol = mp.tile([C, 1], f32, name=f"zcol{b}")
                # P = exp(min_row - E), zcol = rowsum(P); exponents <= 0.
                # P's diagonal is exp(min - ~+147000) == 0 exactly.
                nc.scalar.activation(P_b[:], E_b,
                                     mybir.ActivationFunctionType.Exp,
                                     bias=mcol[:], scale=-1.0,
                                     accum_out=zcol[:])
                rz = mp.tile([C, 1], f32, name=f"rz{b}")
                nc.vector.reciprocal(rz[:], zcol[:])
                scol = mp.tile([C, 1], f32, name=f"scol{b}")
                nc.vector.tensor_tensor(scol[:], rz[:], gcol[:],
                                        op=mybir.AluOpType.mult)
                # attn_s = (gamma/Z) * P + I  -> matmul computes x + gamma*attn@q
                nc.vector.tensor_scalar_mul(P_b[:], P_b[:], scol[:])
                nc.vector.tensor_add(P_b[:], P_b[:], ident[:])
                tp2 = tps.tile([C, C], f32, name=f"tpP{b}", tag="tp")
                nc.tensor.transpose(tp2[:], P_b[:], ident[:])
                attnT = mp.tile([C, C], bf16, name=f"attnT{b}")
                nc.vector.tensor_copy(attnT[:], tp2[:])  # fp32 psum -> bf16
                return attnT

            def emit_apply_chunk(b, attnT, k):
                ost = ostp.tile([C, CHUNK], f16, name=f"ost_{b}_{k}",
                                tag="ost")
                for j in range(CHUNK // OTILE):
                    op = ops.tile([C, OTILE], f32, name=f"op_{b}_{k}_{j}",
                                  tag="op")
                    nc.tensor.matmul(
                        op[:], attnT[:],
                        xb16[b][k][:, j * OTILE:(j + 1) * OTILE],
                        start=True, stop=True)
                    dst = ost[:, j * OTILE:(j + 1) * OTILE]
                    if b == 0:
                        # keep VectorE empty during p2(b0): softmax(b1) must
                        # run on DVE the moment AR(b1) lands, and p2(b0) has
                        # ~24us of slack before that anyway
                        nc.scalar.copy(dst, op[:])
                    elif j % 2 == 0:
                        nc.vector.tensor_copy(dst, op[:])
                    else:
                        nc.scalar.copy(dst, op[:])
                nc.sync.dma_start(o_d[b, :, k * CHUNK:(k + 1) * CHUNK],
                                  ost[:])

            for b in range(B):
                attnT = emit_softmax(b)
                for k in range(NCHUNK):
                    emit_apply_chunk(b, attnT, k)

    _log("tile context done; bacc compile start")
    nc.compile()
    _log("bacc compile done")
    return nc


def _get_nc():
    if "nc" not in _compiled:
        _compiled["nc"] = _build()
    return _compiled["nc"]


def kernel(x, gamma, _trace=False, _tmpdir=None):
    from concourse import bass_utils

    x = np.ascontiguousarray(np.asarray(x), dtype=np.float32)
    gamma = np.asarray(gamma, dtype=np.float32)
    q = x.reshape(B, C, N)
    gcol = np.full((C, 1), gamma[0], dtype=np.float32)
    ident = np.eye(C, dtype=np.float32)

    in_maps = []
    for r in range(NCORES):
        in_maps.append({
            "x": np.ascontiguousarray(q[:, :, r * NLOC:(r + 1) * NLOC]),
            "gamma_col": gcol,
            "ident": ident,
        })

    nc = _get_nc()
    _log("launching run_bass_kernel_spmd")
    res = bass_utils.run_bass_kernel_spmd(
        nc, in_maps, core_ids=list(range(NCORES)), trace=_trace,
        tmpdir=_tmpdir)
    outs = [res.results[r]["out"] for r in range(NCORES)]
    full = np.concatenate(outs, axis=2).astype(np.float32)
    full = full.reshape(B, C, D, H, W)
    if _trace:
        return full.astype(np.float32, copy=False), res
    return full.astype(np.float32, copy=False)


# revision 29
# speedup vs baseline: 1.1217x; 1.1039x over previous
"""Channel-attention module kernel for 8 Trainium2 NeuronCores.

reference semantics (B=2, C=128, N=D*H*W=147456):
    q = x.reshape(B, C, N)
    energy = q @ q^T                  # [B, C, C]
    attn = softmax(rowmax(energy) - energy, axis=-1)
          = softmax(-energy, axis=-1)             (rowmax shift is a no-op)
    out = attn @ q
    return x + gamma * out

Sharding: sequence-parallel over N. Core r owns columns
[r*N/8, (r+1)*N/8) of q for both batches. Each core computes a partial
energy (contraction over its local n), a per-batch AllReduce sums the
tiny [C, C] energy across the 8 cores, each core then computes the
softmax redundantly and applies the attention to its local columns.

Pipelining: energy(b0) -> AR(b0) overlaps energy(b1); AR(b1) overlaps
phase2(b0).

Precision split:
  - energy contraction: true fp32 (softmax argmin gaps as small as 0.03
    on these inputs; one argmin flip alone is ~5% global rel err).
  - phase 2 (attn apply): bf16. The residual is folded into the
    attention matrix (attn_s = gamma/Z * P + I; P's diagonal is exactly
    0 because the energy diagonal ~ +N dominates), so phase 2 is
    out = attn_s @ q with q rounded to bf16 — error is linear, ~0.4%,
    far inside the 2e-2 gate. This makes phase-2 matmuls 4x faster than
    fp32 and lets the fp32 x chunks be freed after phase 1: x lives in
    a small fp32 ring; a resident bf16 copy (cast on the idle ScalarE
    during phase 1) feeds phase 2.
"""

import sys

sys.path.insert(0, "/opt/trn_rl_repo")

import numpy as np

B, C = 2, 128
D, H, W = 16, 96, 96
N = D * H * W  # 147456
NCORES = 8
NLOC = N // NCORES  # 18432
CHUNK = 2048
NCHUNK = NLOC // CHUNK  # 9
OTILE = 512
PIPE = 3  # transposes emitted ahead of their matmul (keeps PE fed)

_compiled = {}


def _log(msg):
    import time as _t
    print(f"[kernel {_t.strftime('%H:%M:%S')}] {msg}", flush=True)


def _build():
    import concourse.bacc as bacc
    import concourse.tile as tile
    import concourse.mybir as mybir

    _log("build start")

    f32 = mybir.dt.float32
    f16 = mybir.dt.float16
    bf16 = mybir.dt.bfloat16
    nc = bacc.Bacc("TRN2", target_bir_lowering=False, debug=False,
                   num_devices=NCORES)

    x_d = nc.dram_tensor("x", [B, C, NLOC], f32, kind="ExternalInput").ap()
    g_d = nc.dram_tensor("gamma_col", [C, 1], f32, kind="ExternalInput").ap()
    id_d = nc.dram_tensor("ident", [C, C], f32, kind="ExternalInput").ap()
    o_d = nc.dram_tensor("out", [B, C, NLOC], f16, kind="ExternalOutput").ap()

    with tile.TileContext(nc) as tc:
        with (
            tc.tile_pool(name="xring", bufs=8) as xp,
            tc.tile_pool(name="xb16", bufs=B * NCHUNK) as xbp,
            tc.tile_pool(name="qt", bufs=6) as qtp,
            tc.tile_pool(name="tps", bufs=3, space="PSUM") as tps,
            tc.tile_pool(name="eps", bufs=2, space="PSUM") as eps,
            tc.tile_pool(name="ops", bufs=3, space="PSUM") as ops,
            tc.tile_pool(name="misc", bufs=1) as mp,
            tc.tile_pool(name="ost", bufs=3) as ostp,
            tc.tile_pool(name="dram", bufs=1, space="DRAM") as dramp,
        ):
            ident = mp.tile([C, C], f32, name="ident_sb")
            nc.sync.dma_start(ident[:], id_d[:])
            # first chunks in consumption-critical order: half of c0, all of
            # c1, rest of c0 — so PE never starves during the DMA ramp
            xt0 = xp.tile([C, CHUNK], f32, name="x_0_0", tag="x")
            xt1 = xp.tile([C, CHUNK], f32, name="x_0_1", tag="x")
            nc.sync.dma_start(xt0[:, 0:1024], x_d[0, :, 0:1024])
            nc.sync.dma_start(xt1[:], x_d[0, :, CHUNK:2 * CHUNK])
            nc.sync.dma_start(xt0[:, 1024:2048], x_d[0, :, 1024:2048])
            pre = {0: xt0, 1: xt1}
            gcol = mp.tile([C, 1], f32, name="gcol")
            nc.sync.dma_start(gcol[:], g_d[:])

            # Warm-up collective: the FIRST collective on this runtime pays
            # a ~45us ncfw cold-start (hw-measured); later ones hit the
            # ~10us floor. Fire a tiny dummy AllReduce immediately so the
            # real per-batch AllReduces run warm.
            w_in = dramp.tile([C, 1], f32, name="w_in")
            w_out = dramp.tile([C, 1], f32, name="w_out", addr_space="Shared")
            nc.gpsimd.dma_start(w_in[:], gcol[:])
            nc.gpsimd.collective_compute(
                "AllReduce", mybir.AluOpType.add,
                replica_groups=[list(range(NCORES))],
                ins=[w_in.opt()], outs=[w_out.opt()],
            )

            xb16 = [[xbp.tile([C, CHUNK], bf16, name=f"xb_{b}_{k}", tag="xb")
                     for k in range(NCHUNK)] for b in range(B)]

            # ---- phase 1 + per-batch AllReduce ----
            ntile_c = CHUNK // C  # 16 n-tiles of 128 per chunk
            ntile = NCHUNK * ntile_c  # 144 per batch
            E_sb = []
            for b in range(B):
                e_ps = eps.tile([C, C], f32, name=f"e_ps{b}", tag="e")
                pend = []
                mm = 0

                def flush(e_ps=e_ps):
                    nonlocal mm
                    qt = pend.pop(0)
                    nc.tensor.matmul(e_ps[:], qt[:], qt[:],
                                     start=(mm == 0), stop=(mm == ntile - 1))
                    mm += 1

                for k in range(NCHUNK):
                    if b == 0 and k <= 1:
                        xt = pre[k]
                    else:
                        xt = xp.tile([C, CHUNK], f32, name=f"x_{b}_{k}",
                                     tag="x")
                        nc.sync.dma_start(
                            xt[:], x_d[b, :, k * CHUNK:(k + 1) * CHUNK])
                    for j in range(ntile_c):
                        t = k * ntile_c + j
                        tp = tps.tile([C, C], f32, name=f"tp_{b}_{t}",
                                      tag="tp")
                        nc.tensor.transpose(
                            tp[:], xt[:, j * C:(j + 1) * C], ident[:])
                        qt = qtp.tile([C, C], f32, name=f"qt_{b}_{t}",
                                      tag="qt")
                        nc.vector.tensor_copy(qt[:], tp[:])
                        pend.append(qt)
                        if len(pend) > PIPE:
                            flush()
                    # bf16 copy for phase 2 (ScalarE is idle in phase 1);
                    # after this the fp32 ring slot can be reused.
                    nc.scalar.copy(xb16[b][k][:], xt[:])
                while pend:
                    flush()
                e_cat = mp.tile([C, C], f32, name=f"e_cat{b}")
                nc.vector.tensor_copy(e_cat[:], e_ps[:])

                ar_in = dramp.tile([C, C], f32, name=f"ar_in{b}")
                ar_out = dramp.tile([C, C], f32, name=f"ar_out{b}",
                                    addr_space="Shared")
                # bounce DMAs on GPSIMD/SWDGE: the HWDGE (sync) ring is
                # strictly FIFO, so a collective-gated load there would
                # block all later chunk loads / output stores.
                nc.gpsimd.dma_start(ar_in[:], e_cat[:])
                nc.gpsimd.collective_compute(
                    "AllReduce", mybir.AluOpType.add,
                    replica_groups=[list(range(NCORES))],
                    ins=[ar_in.opt()], outs=[ar_out.opt()],
                )
                e_red = mp.tile([C, C], f32, name=f"e_red{b}")
                nc.gpsimd.dma_start(e_red[:], ar_out[:])
                E_sb.append(e_red)

            # ---- phase 2: softmax + apply, per batch ----
            def emit_softmax(b):
                E_b = E_sb[b][:]
                mcol = mp.tile([C, 1], f32, name=f"mcol{b}")
                nc.vector.tensor_reduce(mcol[:], E_b, axis=mybir.AxisListType.X,
                                        op=mybir.AluOpType.min)
                P_b = mp.tile([C, C], f32, name=f"P{b}")
                zcol = mp.tile([C, 1], f32, name=f"zcol{b}")
                # P = exp(min_row - E), zcol = rowsum(P); exponents <= 0.
                # P's diagonal is exp(min - ~+147000) == 0 exactly.
                nc.scalar.activation(P_b[:], E_b,
                                     mybir.ActivationFunctionType.Exp,
                                     bias=mcol[:], scale=-1.0,
                                     accum_out=zcol[:])
                rz = mp.tile([C, 1], f32, name=f"rz{b}")
                nc.vector.reciprocal(rz[:], zcol[:])
                scol = mp.tile([C, 1], f32, name=f"scol{b}")
                nc.vector.tensor_tensor(scol[:], rz[:], gcol[:],
                                        op=mybir.AluOpType.mult)
                # attn_s = (gamma/Z) * P + I  -> matmul computes x + gamma*attn@q
                nc.vector.tensor_scalar_mul(P_b[:], P_b[:], scol[:])
                nc.vector.tensor_add(P_b[:], P_b[:], ident[:])
                tp2 = tps.tile([C, C], f32, name=f"tpP{b}", tag="tp")
                nc.tensor.transpose(tp2[:], P_b[:], ident[:])
                attnT = mp.tile([C, C], bf16, name=f"attnT{b}")
                nc.vector.tensor_copy(attnT[:], tp2[:])  # fp32 psum -> bf16
                return attnT

            def emit_apply_chunk(b, attnT, k):
                ost = ostp.tile([C, CHUNK], f16, name=f"ost_{b}_{k}",
                                tag="ost")
                for j in range(CHUNK // OTILE):
                    op = ops.tile([C, OTILE], f32, name=f"op_{b}_{k}_{j}",
                                  tag="op")
                    nc.tensor.matmul(
                        op[:], attnT[:],
                        xb16[b][k][:, j * OTILE:(j + 1) * OTILE],
                        start=True, stop=True)
                    dst = ost[:, j * OTILE:(j + 1) * OTILE]
                    if b == 0:
                        # keep VectorE empty during p2(b0): softmax(b1) must
                        # run on DVE the moment AR(b1) lands, and p2(b0) has
                        # ~24us of slack before that anyway
                        nc.scalar.copy(dst, op[:])
                    elif j % 2 == 0:
                        nc.vector.tensor_copy(dst, op[:])
                    else:
                        nc.scalar.copy(dst, op[:])
                nc.sync.dma_start(o_d[b, :, k * CHUNK:(k + 1) * CHUNK],
                                  ost[:])

            for b in range(B):
                attnT = emit_softmax(b)
                for k in range(NCHUNK):
                    emit_apply_chunk(b, attnT, k)

    _log("tile context done; bacc compile start")
    nc.compile()
    _log("bacc compile done")
    return nc


def _get_nc():
    if "nc" not in _compiled:
        _compiled["nc"] = _build()
    return _compiled["nc"]


def kernel(x, gamma, _trace=False, _tmpdir=None):
    from concourse import bass_utils

    x = np.ascontiguousarray(np.asarray(x), dtype=np.float32)
    gamma = np.asarray(gamma, dtype=np.float32)
    q = x.reshape(B, C, N)
    gcol = np.full((C, 1), gamma[0], dtype=np.float32)
    ident = np.eye(C, dtype=np.float32)

    in_maps = []
    for r in range(NCORES):
        in_maps.append({
            "x": np.ascontiguousarray(q[:, :, r * NLOC:(r + 1) * NLOC]),
            "gamma_col": gcol,
            "ident": ident,
        })

    nc = _get_nc()
    _log("launching run_bass_kernel_spmd")
    res = bass_utils.run_bass_kernel_spmd(
        nc, in_maps, core_ids=list(range(NCORES)), trace=_trace,
        tmpdir=_tmpdir)
    outs = [res.results[r]["out"] for r in range(NCORES)]
    full = np.concatenate(outs, axis=2).astype(np.float32)
    full = full.reshape(B, C, D, H, W)
    if _trace:
        return full.astype(np.float32, copy=False), res
    return full.astype(np.float32, copy=False)
